# revision 2
# baseline (speedup 1.0000x reference)
"""Trainium2 Bass kernel: out = x * w  (per-column scale, broadcast over rows).

x: (131072, 1024) f32, w: (1024,) f32. Sharded row-wise across 8 NeuronCores;
each core handles 16384 rows (64 MiB in + 64 MiB out -> HBM-bound,
roofline ~375 us/core at ~358 GB/s).

Layout per core: rows r = n*512 + p*4 + g  ->  view [p=128, n=32, (g d)=4096].
Each partition line is 16 KiB contiguous DRAM; one dma_start moves a 2 MiB
tile (above the ~1 MiB knee for >=75% DMA efficiency). Loads issue on the
sync (SP) HWDGE ring, stores on the scalar (ACT) ring so they never FIFO-block
each other. The multiply is one fp32 tensor_tensor per tile on DVE (~137 us
total, under the DMA span). w is broadcast once into SBUF across all 128
partitions with a stride-0 DMA.
"""

import sys

if "/opt/trn_rl_repo" not in sys.path:
    sys.path.insert(0, "/opt/trn_rl_repo")

import numpy as np

N, D = 131072, 1024
NCORES = 8
ROWS = N // NCORES          # 16384 rows per core
P = 128                     # SBUF partitions
G = 4                       # rows per partition per tile
F = G * D                   # free elems per partition per tile (16 KiB)
TILE_ROWS = P * G           # 512 rows per tile
NTILES = ROWS // TILE_ROWS  # 32 tiles per core

_built = {}


def _build():
    if "nc" in _built:
        return _built["nc"]

    import concourse.bass as bass  # noqa: F401
    from concourse import bacc, mybir, tile

    f32 = mybir.dt.float32
    nc = bacc.Bacc(
        "TRN2", target_bir_lowering=False, debug=False, num_devices=NCORES
    )

    x = nc.dram_tensor("x", [ROWS, D], f32, kind="ExternalInput").ap()
    w = nc.dram_tensor("w", [D], f32, kind="ExternalInput").ap()
    out = nc.dram_tensor("out", [ROWS, D], f32, kind="ExternalOutput").ap()

    xv = x.rearrange("(n p g) d -> p n (g d)", p=P, g=G)
    ov = out.rearrange("(n p g) d -> p n (g d)", p=P, g=G)

    with tile.TileContext(nc) as tc:
        with (
            tc.tile_pool(name="wp", bufs=1) as wp,
            tc.tile_pool(name="inp", bufs=4) as inp,
            tc.tile_pool(name="outp", bufs=4) as outp,
        ):
            wt = wp.tile([P, F], f32)
            wsrc = w.unsqueeze(0).unsqueeze(0).broadcast_to([P, G, D])
            nc.sync.dma_start(
                wt[:].rearrange("p (g d) -> p g d", d=D), wsrc
            )
            for t in range(NTILES):
                xt = inp.tile([P, F], f32)
                nc.sync.dma_start(xt[:], xv[:, t, :])
                ot = outp.tile([P, F], f32)
                nc.vector.tensor_mul(ot[:], xt[:], wt[:])
                nc.scalar.dma_start(ov[:, t, :], ot[:])

    nc.compile()
    _built["nc"] = nc
    return nc


def _run(x: np.ndarray, w: np.ndarray, **kw):
    """Shard, execute on 8 cores, return (full_output, BassKernelResults)."""
    from concourse import bass_utils

    nc = _build()
    x = np.ascontiguousarray(x, dtype=np.float32)
    w = np.ascontiguousarray(w, dtype=np.float32)

    in_maps = [
        {"x": x[i * ROWS : (i + 1) * ROWS], "w": w} for i in range(NCORES)
    ]
    res = bass_utils.run_bass_kernel_spmd(nc, in_maps, list(range(NCORES)), **kw)
    out = np.concatenate([r["out"] for r in res.results], axis=0)
    return out, res


def kernel(x: np.ndarray, w: np.ndarray) -> np.ndarray:
    return _run(x, w)[0]
